# revision 1
# baseline (speedup 1.0000x reference)
"""Trainium2 Bass kernel for nn_Block_24343874633736 (moe_routing).

Transformer block: RMSNorm -> MHA(RoPE) -> residual -> RMSNorm ->
MoE (8 routed experts, top-2, + 1 shared expert) -> residual.

Sharding (8 NeuronCores, single SPMD launch):
  - Attention: data-parallel over tokens. Core c owns 512 query rows of
    batch c//4. K/V are computed for the local 512 rows only, then
    AllGather'd (transposed K and natural V) within each batch's
    4-core group.
  - MoE: expert-parallel, one routed expert per core. hn activations and
    fp32 gate logits are AllGather'd across all 8 cores; every core
    computes the full top-2 routing mask, compacts the token list for
    ITS expert (per-tile triangular-matmul cumsum + batched cross-tile
    exclusive cumsum of tile totals -- no serial chain), scatters rows by
    indirect DMA, runs the expert FFN at bf16, and returns raw expert
    rows + (weight, token) pairs. The host scatter-adds.
  - Shared expert + residuals: token-sharded; emitted between the
    collectives and routing so its TensorE work fills that window.

Numerics: bf16 TensorE matmuls with fp32 PSUM accumulation everywhere
except the gate logits, which are computed in fp32 from fp32 h so the
top-2 selection matches the fp32 reference (bf16 logits flip ~30
near-tie tokens and each flip is a large localized error).
"""

import sys

for _p in ("/opt/trn_rl_repo",):
    if _p not in sys.path:
        sys.path.insert(0, _p)

import numpy as np
import ml_dtypes

import concourse.bass as bass
import concourse.mybir as mybir
from concourse import bacc
from concourse.masks import make_identity, make_upper_triangular
from concourse.tile import TileContext

BF16 = ml_dtypes.bfloat16
F32 = mybir.dt.float32
BF = mybir.dt.bfloat16
I32 = mybir.dt.int32
AX = mybir.AxisListType
OP = mybir.AluOpType
ACTF = mybir.ActivationFunctionType

P = 128
DIM = 1024
NH = 16
HD = 64
E = 8
HID = 1024
EPS = 1e-6
BIG = 60000.0  # trash slot index (> any capacity; exact in fp32/int32)

B_FULL, S_FULL = 2, 2048
LQ_FULL = 512           # tokens owned per core
C_FULL = 1280           # per-expert token capacity (max observed 1109)


def _ts(i, n):
    return slice(i * n, (i + 1) * n)


def build_nc(LQ=LQ_FULL, C=C_FULL, n_cores=8, n_batch=B_FULL):
    """Emit the SPMD Bass program. All 8 cores run this same program."""
    G = n_cores // n_batch   # cores per batch group
    LT = LQ * G              # K/V length per batch
    NT = LT // P
    NQ = LQ // P
    NCAP = C // P
    ND = DIM // P
    N_ALL = LQ * n_cores
    NA = N_ALL // P
    assert NA <= P

    nc = bacc.Bacc("TRN2", target_bir_lowering=False, debug=False,
                   num_devices=n_cores)

    # ---- I/O (per-core slices are carved host-side) ----
    x_in = nc.dram_tensor("x_chunk", [LQ, DIM], F32, kind="ExternalInput")
    cos_in = nc.dram_tensor("cos_chunk", [LQ, DIM // 2], F32, kind="ExternalInput")
    sin_in = nc.dram_tensor("sin_chunk", [LQ, DIM // 2], F32, kind="ExternalInput")
    wq_in = nc.dram_tensor("wq_bf", [DIM, DIM], BF, kind="ExternalInput")
    wk_in = nc.dram_tensor("wk_bf", [DIM, DIM], BF, kind="ExternalInput")
    wv_in = nc.dram_tensor("wv_bf", [DIM, DIM], BF, kind="ExternalInput")
    wo_in = nc.dram_tensor("wo_bf", [DIM, DIM], BF, kind="ExternalInput")
    gate_in = nc.dram_tensor("gate32", [DIM, E], F32, kind="ExternalInput")
    sw1_in = nc.dram_tensor("sw1_bf", [DIM, HID], BF, kind="ExternalInput")
    sw2_in = nc.dram_tensor("sw2_bf", [HID, DIM], BF, kind="ExternalInput")
    sw3_in = nc.dram_tensor("sw3_bf", [DIM, HID], BF, kind="ExternalInput")
    ew1_in = nc.dram_tensor("ew1_bf", [DIM, HID], BF, kind="ExternalInput")
    ew2_in = nc.dram_tensor("ew2_bf", [HID, DIM], BF, kind="ExternalInput")
    ew3_in = nc.dram_tensor("ew3_bf", [DIM, HID], BF, kind="ExternalInput")
    oh_in = nc.dram_tensor("onehot", [1, NA * E], F32, kind="ExternalInput")

    out_local = nc.dram_tensor("out_local", [LQ, DIM], F32, kind="ExternalOutput")
    eo_out = nc.dram_tensor("eo_out", [C, DIM], F32, kind="ExternalOutput")
    lg_out = nc.dram_tensor("lg_out", [N_ALL, E], F32, kind="ExternalOutput")

    # internal DRAM (collective bounce + expert staging)
    kT_loc = nc.dram_tensor("kT_loc", [DIM, LQ], BF)
    v_loc = nc.dram_tensor("v_loc", [LQ, DIM], BF)
    kT_full = nc.dram_tensor("kT_full", [G * DIM, LQ], BF)
    v_full = nc.dram_tensor("v_full", [LT, DIM], BF)
    # hl_loc cols [0:DIM//2) f32 = hn bitcast; cols [DIM//2:DIM//2+E) = fp32 logits
    HLC = DIM // 2 + E
    hl_loc = nc.dram_tensor("hl_loc", [LQ, HLC], F32)
    hl_full = nc.dram_tensor("hl_full", [N_ALL, HLC], F32, addr_space="Shared")
    NSPLIT = 4  # scatter-chain split: breaks the WAW serialization
    ebufs = [nc.dram_tensor(f"ebuf{i}", [C, DIM], BF) for i in range(NSPLIT)]

    kv_groups = [list(range(g * G, (g + 1) * G)) for g in range(n_batch)]
    all_groups = [list(range(n_cores))]

    from contextlib import ExitStack
    with TileContext(nc) as tc, ExitStack() as stack:
        const_pool = stack.enter_context(tc.tile_pool(name="const", bufs=1))
        id_bf = const_pool.tile([P, P], BF)
        make_identity(nc, id_bf[:])
        id_f32 = const_pool.tile([P, P], F32)
        make_identity(nc, id_f32[:])
        ltri = const_pool.tile([P, P], F32)
        make_upper_triangular(nc, ltri[:], val=1.0, diag=True)  # L[k,p]=1 iff k<=p
        ltri_s = const_pool.tile([P, P], F32)
        make_upper_triangular(nc, ltri_s[:], val=1.0, diag=False)  # k<p
        ones_col = const_pool.tile([P, 1], F32)
        nc.vector.memset(ones_col[:], 1.0)
        eps_col = const_pool.tile([P, 1], F32)
        nc.vector.memset(eps_col[:], EPS)
        oh_bc = const_pool.tile([P, NA * E], F32)
        oh_row = const_pool.tile([1, NA * E], F32)
        nc.sync.dma_start(out=oh_row[:], in_=oh_in[:, :])
        nc.gpsimd.partition_broadcast(oh_bc[:], oh_row[:])

        # persistent activations (whole-kernel lifetime)
        persist = stack.enter_context(tc.tile_pool(name="persist", bufs=1))
        h_sb = [persist.tile([P, DIM], F32, name=f"h{i}", tag=f"h{i}")
                for i in range(NQ)]
        hnT = [persist.tile([P, LQ], BF, name=f"hnT{j}", tag=f"hnT{j}")
               for j in range(ND)]

        # LIFO-nested scoped pools: attT (close after D) > kv (after C) > xnT (after B)
        sc_attT = ExitStack()
        p_attT = sc_attT.enter_context(tc.tile_pool(name="p_attT", bufs=1))
        sc_kv = ExitStack()
        p_kv = sc_kv.enter_context(tc.tile_pool(name="p_kv", bufs=1))
        sc_xnT = ExitStack()
        p_xnT = sc_xnT.enter_context(tc.tile_pool(name="p_xnT", bufs=1))

        # =============== stage A: rmsnorm + xn^T (local rows) ========
        scA = nc.enter_named_scope("A_norm", False)
        xnT = [p_xnT.tile([P, LQ], BF, name=f"xnT{j}", tag=f"xnT{j}")
               for j in range(ND)]
        with tc.tile_pool(name="stA", bufs=2) as pa, \
             tc.tile_pool(name="stA_ps", bufs=4, space="PSUM") as pa_ps:
            for t in range(NQ):
                xt = pa.tile([P, DIM], F32, tag="xt")
                nc.sync.dma_start(out=xt[:], in_=x_in[_ts(t, P), :])
                sq = pa.tile([P, DIM], F32, tag="sq")
                ssq = pa.tile([P, 1], F32, tag="ssq")
                nc.scalar.activation(out=sq[:], in_=xt[:], func=ACTF.Square,
                                     accum_out=ssq[:])
                rms = pa.tile([P, 1], F32, tag="rms")
                nc.scalar.activation(out=rms[:], in_=ssq[:], func=ACTF.Sqrt,
                                     scale=1.0 / DIM, bias=eps_col[:])
                rr = pa.tile([P, 1], F32, tag="rr")
                nc.vector.reciprocal(out=rr[:], in_=rms[:])
                xn = pa.tile([P, DIM], BF, tag="xn")
                nc.scalar.activation(out=xn[:], in_=xt[:], func=ACTF.Copy,
                                     scale=rr[:])
                for j in range(ND):
                    pst = pa_ps.tile([P, P], BF, space="PSUM", tag="pst")
                    nc.tensor.transpose(out=pst[:], in_=xn[:, _ts(j, P)],
                                        identity=id_bf[:])
                    nc.vector.tensor_copy(out=xnT[j][:, _ts(t, P)], in_=pst[:])
        nc.leave_named_scope("A_norm", scA[0], False)

        # =============== stage B: Q/K/V local + KV AllGather =========
        scB = nc.enter_named_scope("B_qkv", False)
        kT = [p_kv.tile([P, LT], BF, name=f"kT{j}", tag=f"kT{j}")
              for j in range(ND)]
        vaug = [p_kv.tile([P, NH * (HD + 1)], BF, name=f"va{t}", tag=f"va{t}")
                for t in range(NT)]
        qT = [p_kv.tile([P, LQ], BF, name=f"qT{j}", tag=f"qT{j}")
              for j in range(ND)]

        def load_w(pool, src, tag, slot=None):
            """Weight loads; same `slot` reuses the same SBUF slots sequentially."""
            slot = slot or tag
            w = [pool.tile([P, DIM], BF, name=f"{tag}{j}", tag=f"{slot}{j}")
                 for j in range(ND)]
            for j in range(ND):
                nc.sync.dma_start(out=w[j][:], in_=src[_ts(j, P), :])
            return w

        def proj(ps_pool, w_sb, t):
            ps = ps_pool.tile([P, DIM], F32, space="PSUM", tag="proj")
            for half in range(2):
                for j in range(ND):
                    nc.tensor.matmul(
                        out=ps[:, _ts(half, 512)],
                        lhsT=xnT[j][:, _ts(t, P)],
                        rhs=w_sb[j][:, _ts(half, 512)],
                        start=(j == 0), stop=(j == ND - 1))
            return ps

        def rope_to(pool, ps, cs, sn, out_tag):
            o = pool.tile([P, DIM], BF, tag=out_tag)
            t1 = pool.tile([P, DIM // 2], F32, tag="rp1")
            t2 = pool.tile([P, DIM // 2], F32, tag="rp2")
            nc.vector.tensor_tensor(out=t1[:], in0=ps[:, 0::2], in1=cs[:], op=OP.mult)
            nc.vector.tensor_tensor(out=t2[:], in0=ps[:, 1::2], in1=sn[:], op=OP.mult)
            nc.vector.tensor_tensor(out=o[:, 0::2], in0=t1[:], in1=t2[:], op=OP.subtract)
            nc.vector.tensor_tensor(out=t1[:], in0=ps[:, 0::2], in1=sn[:], op=OP.mult)
            nc.vector.tensor_tensor(out=t2[:], in0=ps[:, 1::2], in1=cs[:], op=OP.mult)
            nc.vector.tensor_tensor(out=o[:, 1::2], in0=t1[:], in1=t2[:], op=OP.add)
            return o

        with tc.tile_pool(name="stB", bufs=3) as pb, \
             tc.tile_pool(name="stB_w", bufs=1) as pw, \
             tc.tile_pool(name="stB_ps", bufs=2, space="PSUM") as pb_ps, \
             tc.tile_pool(name="stB_pst", bufs=4, space="PSUM") as pb_pst:
            wk_sb = load_w(pw, wk_in, "wk", slot="watt")
            for t in range(NQ):
                cs = pb.tile([P, DIM // 2], F32, tag="cs")
                sn = pb.tile([P, DIM // 2], F32, tag="sn")
                nc.sync.dma_start(out=cs[:], in_=cos_in[_ts(t, P), :])
                nc.sync.dma_start(out=sn[:], in_=sin_in[_ts(t, P), :])
                ps = proj(pb_ps, wk_sb, t)
                kr = rope_to(pb, ps, cs, sn, "kr")
                for j in range(ND):
                    pst = pb_pst.tile([P, P], BF, space="PSUM", tag="pstB")
                    nc.tensor.transpose(out=pst[:], in_=kr[:, _ts(j, P)],
                                        identity=id_bf[:])
                    kc = pb.tile([P, P], BF, tag="kc")
                    nc.vector.tensor_copy(out=kc[:], in_=pst[:])
                    nc.sync.dma_start(out=kT_loc[_ts(j, P), _ts(t, P)], in_=kc[:])
            # K gather starts while V/Q projections run
            nc.gpsimd.collective_compute(
                "AllGather", OP.bypass, replica_groups=kv_groups,
                ins=[kT_loc.ap().opt()], outs=[kT_full.ap().opt()])
            wv_sb = load_w(pw, wv_in, "wv", slot="watt")
            for t in range(NQ):
                ps = proj(pb_ps, wv_sb, t)
                vc = pb.tile([P, DIM], BF, tag="vc")
                nc.vector.tensor_copy(out=vc[:], in_=ps[:])
                nc.sync.dma_start(out=v_loc[_ts(t, P), :], in_=vc[:])
            nc.gpsimd.collective_compute(
                "AllGather", OP.bypass, replica_groups=kv_groups,
                ins=[v_loc.ap().opt()], outs=[v_full.ap().opt()])
            wq_sb = load_w(pw, wq_in, "wq", slot="watt")
            for t in range(NQ):
                cs = pb.tile([P, DIM // 2], F32, tag="cs")
                sn = pb.tile([P, DIM // 2], F32, tag="sn")
                nc.sync.dma_start(out=cs[:], in_=cos_in[_ts(t, P), :])
                nc.sync.dma_start(out=sn[:], in_=sin_in[_ts(t, P), :])
                ps = proj(pb_ps, wq_sb, t)
                qr = rope_to(pb, ps, cs, sn, "qr")
                for j in range(ND):
                    pst = pb_pst.tile([P, P], BF, space="PSUM", tag="pstB")
                    nc.tensor.transpose(out=pst[:], in_=qr[:, _ts(j, P)],
                                        identity=id_bf[:])
                    nc.vector.tensor_copy(out=qT[j][:, _ts(t, P)], in_=pst[:])

            # pull gathered K^T and V into SBUF
            kTf = kT_full.ap().rearrange("(g d) q -> g d q", g=G)
            for j in range(ND):
                for g in range(G):
                    nc.sync.dma_start(out=kT[j][:, _ts(g, LQ)],
                                      in_=kTf[g, _ts(j, P), :])
            vf = v_full.ap().rearrange("(n p) (h d) -> n p h d", p=P, h=NH)
            for t in range(NT):
                va_v = vaug[t][:].rearrange("p (h d) -> p h d", h=NH)
                nc.sync.dma_start(out=va_v[:, :, 0:HD], in_=vf[t])
                nc.vector.memset(va_v[:, :, HD:HD + 1], 1.0)
        sc_xnT.close()
        # zero the ebuf split buffers (scalar-engine DMA queue; must finish
        # before stage F's scatters, which is ~400us later)
        with tc.tile_pool(name="ez", bufs=1) as pez:
            zt = pez.tile([P, DIM], BF)
            nc.vector.memset(zt[:], 0.0)
            for i in range(NSPLIT):
                for sc_ in range(NCAP):
                    nc.scalar.dma_start(out=ebufs[i][_ts(sc_, P), :], in_=zt[:])
        nc.leave_named_scope("B_qkv", scB[0], False)

        # =============== stage C: attention core =====================
        scC = nc.enter_named_scope("C_attn", False)
        attT = [p_attT.tile([P, LQ], BF, name=f"attT{j}", tag=f"attT{j}")
                for j in range(ND)]
        with tc.tile_pool(name="stC", bufs=6) as pc, \
             tc.tile_pool(name="stC_ps", bufs=3, space="PSUM") as pc_ps, \
             tc.tile_pool(name="stC_av", bufs=2, space="PSUM") as pc_av:
            for h in range(NH):
                jj, sub = h // 2, h % 2
                kT_h = kT[jj][_ts(sub, HD), :]
                qT_h = qT[jj][_ts(sub, HD), :]
                expT = []
                for tg in range(NT // 2):
                    sps = pc_ps.tile([P, 2 * LQ], F32, space="PSUM", tag="scores")
                    for u in range(2):
                        nc.tensor.matmul(out=sps[:, _ts(u, LQ)],
                                         lhsT=kT_h[:, _ts(2 * tg + u, P)],
                                         rhs=qT_h[:, :], start=True, stop=True)
                    ex = pc.tile([P, 2 * LQ], BF, tag="expT")
                    nc.scalar.activation(out=ex[:], in_=sps[:], func=ACTF.Exp)
                    expT.append(ex)
                aug = pc_av.tile([HD + 1, LQ], F32, space="PSUM", tag="aug")
                for t in range(NT):
                    nc.tensor.matmul(
                        out=aug[:],
                        lhsT=vaug[t][:, h * (HD + 1):(h + 1) * (HD + 1)],
                        rhs=expT[t // 2][:, _ts(t % 2, LQ)],
                        start=(t == 0), stop=(t == NT - 1))
                rcp = pc.tile([1, LQ], F32, tag="rcp")
                nc.vector.reciprocal(out=rcp[:], in_=aug[HD:HD + 1, :])
                rbc = pc.tile([HD, LQ], F32, tag="rbc")
                nc.gpsimd.partition_broadcast(rbc[:], rcp[:])
                nc.vector.tensor_tensor(out=attT[jj][_ts(sub, HD), :],
                                        in0=aug[0:HD, :], in1=rbc[:], op=OP.mult)
        sc_kv.close()
        nc.leave_named_scope("C_attn", scC[0], False)

        # =============== stage D: O-proj, gate, hn ===================
        scD = nc.enter_named_scope("D_oproj", False)
        with tc.tile_pool(name="stD", bufs=3) as pd, \
             tc.tile_pool(name="stD_w", bufs=1) as pdw, \
             tc.tile_pool(name="stD_ps", bufs=2, space="PSUM") as pd_ps, \
             tc.tile_pool(name="stD_gps", bufs=2, space="PSUM") as pd_gps, \
             tc.tile_pool(name="stD_pst", bufs=2, space="PSUM") as pd_pst, \
             tc.tile_pool(name="stD_hT", bufs=1) as pd_hT:
            wo_sb = load_w(pdw, wo_in, "wo")
            gate_sb = [pdw.tile([P, E], F32, name=f"g32_{j}", tag=f"g32_{j}")
                       for j in range(ND)]
            for j in range(ND):
                nc.sync.dma_start(out=gate_sb[j][:], in_=gate_in[_ts(j, P), :])
            hT32 = [pd_hT.tile([P, LQ], F32, name=f"hT{j}", tag=f"hT{j}")
                    for j in range(ND)]
            for t in range(NQ):
                ps = pd_ps.tile([P, DIM], F32, space="PSUM", tag="ops")
                for half in range(2):
                    for j in range(ND):
                        nc.tensor.matmul(
                            out=ps[:, _ts(half, 512)],
                            lhsT=attT[j][:, _ts(t, P)],
                            rhs=wo_sb[j][:, _ts(half, 512)],
                            start=(j == 0), stop=(j == ND - 1))
                xres = pd.tile([P, DIM], F32, tag="xres")
                nc.sync.dma_start(out=xres[:], in_=x_in[_ts(t, P), :])
                nc.vector.tensor_tensor(out=h_sb[t][:], in0=ps[:],
                                        in1=xres[:], op=OP.add)
                for j in range(ND):
                    pst = pd_pst.tile([P, P], F32, space="PSUM", tag="pstD")
                    nc.tensor.transpose(out=pst[:], in_=h_sb[t][:, _ts(j, P)],
                                        identity=id_f32[:])
                    nc.vector.tensor_copy(out=hT32[j][:, _ts(t, P)], in_=pst[:])
            for t in range(NQ):
                # fp32 gate logits = (h @ gate_eff) * r2
                gps = pd_gps.tile([P, E], F32, space="PSUM", tag="gps")
                for j in range(ND):
                    nc.tensor.matmul(out=gps[:], lhsT=hT32[j][:, _ts(t, P)],
                                     rhs=gate_sb[j][:],
                                     start=(j == 0), stop=(j == ND - 1))
                sq = pd.tile([P, DIM], F32, tag="sqD")
                ssq = pd.tile([P, 1], F32, tag="ssqD")
                nc.scalar.activation(out=sq[:], in_=h_sb[t][:], func=ACTF.Square,
                                     accum_out=ssq[:])
                rms = pd.tile([P, 1], F32, tag="rmsD")
                nc.scalar.activation(out=rms[:], in_=ssq[:], func=ACTF.Sqrt,
                                     scale=1.0 / DIM, bias=eps_col[:])
                rr = pd.tile([P, 1], F32, tag="rrD")
                nc.vector.reciprocal(out=rr[:], in_=rms[:])
                lg = pd.tile([P, E], F32, tag="lg")
                nc.vector.tensor_scalar_mul(lg[:], gps[:], rr[:])
                nc.sync.dma_start(out=hl_loc[_ts(t, P), DIM // 2:], in_=lg[:])
                hn = pd.tile([P, DIM], BF, tag="hnD")
                nc.scalar.activation(out=hn[:], in_=h_sb[t][:], func=ACTF.Copy,
                                     scale=rr[:])
                hl_bf = hl_loc.ap().bitcast(BF)
                nc.sync.dma_start(out=hl_bf[_ts(t, P), 0:DIM], in_=hn[:])
                for j in range(ND):
                    pst = pd_pst.tile([P, P], BF, space="PSUM", tag="pstD")
                    nc.tensor.transpose(out=pst[:], in_=hn[:, _ts(j, P)],
                                        identity=id_bf[:])
                    nc.vector.tensor_copy(out=hnT[j][:, _ts(t, P)], in_=pst[:])
        sc_attT.close()
        nc.leave_named_scope("D_oproj", scD[0], False)

        # =============== collectives (hn + logits) ===================
        scCC = nc.enter_named_scope("CC_gather", False)
        nc.gpsimd.collective_compute(
            "AllGather", OP.bypass, replica_groups=all_groups,
            ins=[hl_loc.ap().opt()], outs=[hl_full.ap().opt()])
        nc.leave_named_scope("CC_gather", scCC[0], False)

        # =============== stage H: shared expert + local output =======
        # (emitted before routing: no dependency on the collectives, so its
        # TensorE work fills the gather + routing window)
        scH = nc.enter_named_scope("H_shared", False)
        with tc.tile_pool(name="stH", bufs=3) as ph, \
             tc.tile_pool(name="stH_w", bufs=1) as phw, \
             tc.tile_pool(name="stH_gT", bufs=1) as ph_gT, \
             tc.tile_pool(name="stH_ps", bufs=2, space="PSUM") as ph_ps:
            s1_sb = load_w(phw, sw1_in, "s1")
            s3_sb = load_w(phw, sw3_in, "s3")
            gsT = [ph_gT.tile([P, LQ], BF, name=f"gsT{j}", tag=f"gsT{j}")
                   for j in range(ND)]
            for j in range(ND):
                h1 = ph_ps.tile([P, LQ], F32, space="PSUM", tag="sh1")
                h3 = ph_ps.tile([P, LQ], F32, space="PSUM", tag="sh3")
                for d in range(ND):
                    nc.tensor.matmul(out=h1[:], lhsT=s1_sb[d][:, _ts(j, P)],
                                     rhs=hnT[d][:, :],
                                     start=(d == 0), stop=(d == ND - 1))
                for d in range(ND):
                    nc.tensor.matmul(out=h3[:], lhsT=s3_sb[d][:, _ts(j, P)],
                                     rhs=hnT[d][:, :],
                                     start=(d == 0), stop=(d == ND - 1))
                sig = ph.tile([P, LQ], F32, tag="sigH")
                nc.scalar.activation(out=sig[:], in_=h1[:], func=ACTF.Sigmoid)
                nc.vector.tensor_tensor(out=sig[:], in0=sig[:], in1=h1[:],
                                        op=OP.mult)
                nc.vector.tensor_tensor(out=gsT[j][:], in0=sig[:], in1=h3[:],
                                        op=OP.mult)
            s2_sb = load_w(phw, sw2_in, "s2", slot="s1")
            for t in range(NQ):
                ps = ph_ps.tile([P, DIM], F32, space="PSUM", tag="shps")
                for half in range(2):
                    for j in range(ND):
                        nc.tensor.matmul(
                            out=ps[:, _ts(half, 512)],
                            lhsT=gsT[j][:, _ts(t, P)],
                            rhs=s2_sb[j][:, _ts(half, 512)],
                            start=(j == 0), stop=(j == ND - 1))
                ot = ph.tile([P, DIM], F32, tag="ot")
                nc.vector.tensor_tensor(out=ot[:], in0=ps[:], in1=h_sb[t][:],
                                        op=OP.add)
                nc.sync.dma_start(out=out_local[_ts(t, P), :], in_=ot[:])
        nc.leave_named_scope("H_shared", scH[0], False)

        # =============== stage F: routing + dispatch =================
        # Selection happens on raw fp32 logits (monotonic-invariant), so the
        # host can replicate slot assignment exactly from lg_out; routing
        # weights and token indices are recovered host-side. The device only
        # compacts hn rows for its expert.
        scF = nc.enter_named_scope("F_route", False)
        with tc.tile_pool(name="stF", bufs=8) as pf, \
             tc.tile_pool(name="stF_keep", bufs=1) as pfk, \
             tc.tile_pool(name="stF_ps", bufs=2, space="PSUM") as pf_ps, \
             tc.tile_pool(name="stF_tot", bufs=1, space="PSUM") as pf_tot:
            hl_bf_full = hl_full.ap().bitcast(BF)
            lg_all = pfk.tile([P, NA * E], F32)
            for t in range(NA):
                nc.sync.dma_start(out=lg_all[:, _ts(t, E)],
                                  in_=hl_full[_ts(t, P), DIM // 2:])
            nc.sync.dma_start(
                out=lg_out.ap().rearrange("(t p) e -> p t e", p=P),
                in_=lg_all[:].rearrange("p (t e) -> p t e", t=NA))
            v3 = lg_all[:].rearrange("p (t e) -> p t e", t=NA)
            m1 = pfk.tile([P, NA], F32)
            nc.vector.reduce_max(out=m1[:], in_=v3, axis=AX.X)
            ge1 = pfk.tile([P, NA * E], F32)
            g13 = ge1[:].rearrange("p (t e) -> p t e", t=NA)
            nc.vector.tensor_tensor(out=g13, in0=v3,
                                    in1=m1[:, :, None].to_broadcast([P, NA, E]),
                                    op=OP.is_ge)
            msk = pfk.tile([P, NA * E], F32)
            nc.vector.tensor_scalar_mul(msk[:], ge1[:], -1.0e30)
            nc.vector.tensor_tensor(out=msk[:], in0=msk[:], in1=lg_all[:],
                                    op=OP.add)
            m2 = pfk.tile([P, NA], F32)
            nc.vector.reduce_max(out=m2[:],
                                 in_=msk[:].rearrange("p (t e) -> p t e", t=NA),
                                 axis=AX.X)
            ge = pfk.tile([P, NA * E], F32)
            ge3 = ge[:].rearrange("p (t e) -> p t e", t=NA)
            nc.vector.tensor_tensor(out=ge3, in0=v3,
                                    in1=m2[:, :, None].to_broadcast([P, NA, E]),
                                    op=OP.is_ge)
            msel = pfk.tile([P, NA * E], F32)
            nc.vector.tensor_tensor(out=msel[:], in0=ge[:], in1=oh_bc[:],
                                    op=OP.mult)
            ind = pfk.tile([P, NA], F32)
            nc.vector.reduce_sum(out=ind[:],
                                 in_=msel[:].rearrange("p (t e) -> p t e", t=NA),
                                 axis=AX.X)
            # per-tile totals + within-tile inclusive cumsum: one matmul each
            tots = pf_tot.tile([1, NA], F32, space="PSUM")
            nc.tensor.matmul(out=tots[:], lhsT=ones_col[:], rhs=ind[:],
                             start=True, stop=True)
            cnts = pf_tot.tile([P, NA], F32, space="PSUM")
            nc.tensor.matmul(out=cnts[:], lhsT=ltri[:], rhs=ind[:],
                             start=True, stop=True)
            # batched exclusive cumsum of tile totals -> per-tile bases
            tots_sb = pf.tile([1, NA], F32, tag="tots_sb")
            nc.vector.tensor_copy(out=tots_sb[:], in_=tots[:])
            totsT_ps = pf_ps.tile([NA, 1], F32, space="PSUM", tag="totsT", bufs=1)
            nc.tensor.transpose(out=totsT_ps[:], in_=tots_sb[:],
                                identity=id_f32[:1, :1])
            totsT = pf.tile([NA, 1], F32, tag="totsT_sb")
            nc.vector.tensor_copy(out=totsT[:], in_=totsT_ps[:])
            basesT_ps = pf_ps.tile([NA, 1], F32, space="PSUM", tag="basesT", bufs=1)
            nc.tensor.matmul(out=basesT_ps[:], lhsT=ltri_s[:NA, :NA],
                             rhs=totsT[:], start=True, stop=True)
            basesT = pf.tile([NA, 1], F32, tag="basesT_sb")
            nc.vector.tensor_copy(out=basesT[:], in_=basesT_ps[:])
            bases_ps = pf_ps.tile([1, NA], F32, space="PSUM", tag="bases", bufs=1)
            nc.tensor.transpose(out=bases_ps[:], in_=basesT[:],
                                identity=id_f32[:NA, :NA])
            bases_sb = pf.tile([1, NA], F32, tag="bases_sb")
            nc.vector.tensor_copy(out=bases_sb[:], in_=bases_ps[:])
            bb_all = pfk.tile([P, NA], F32)
            nc.gpsimd.partition_broadcast(bb_all[:], bases_sb[:])
            # destinations (batched)
            d_all = pfk.tile([P, NA], F32)
            nc.vector.scalar_tensor_tensor(
                out=d_all[:], in0=cnts[:], scalar=-(1.0 + BIG),
                in1=bb_all[:], op0=OP.add, op1=OP.add)
            nc.vector.tensor_tensor(out=d_all[:], in0=d_all[:], in1=ind[:],
                                    op=OP.mult)
            nc.vector.tensor_scalar_add(d_all[:], d_all[:], BIG)
            dest_all = pfk.tile([P, NA], I32)
            nc.vector.tensor_copy(out=dest_all[:], in_=d_all[:])
            # scatters (independent per tile)
            for t in range(NA):
                hnt = pf.tile([P, DIM], BF, tag="hnF")
                nc.sync.dma_start(out=hnt[:], in_=hl_bf_full[_ts(t, P), 0:DIM])
                nc.gpsimd.indirect_dma_start(
                    out=ebufs[t % NSPLIT][:, :],
                    out_offset=bass.IndirectOffsetOnAxis(
                        ap=dest_all[:, t:t + 1], axis=0),
                    in_=hnt[:], in_offset=None,
                    bounds_check=C - 1, oob_is_err=False)
        nc.leave_named_scope("F_route", scF[0], False)

        # =============== stage G: expert FFN =========================
        scG = nc.enter_named_scope("G_expert", False)
        with tc.tile_pool(name="stG", bufs=3) as pg, \
             tc.tile_pool(name="stG_w", bufs=1) as pgw, \
             tc.tile_pool(name="stG_gT", bufs=1) as pg_gT:
            ebT = [pg_gT.tile([P, C], BF, name=f"ebT{j}", tag=f"ebT{j}")
                   for j in range(ND)]
            with tc.tile_pool(name="stG_ps", bufs=4, space="PSUM") as pg_ps:
                for s in range(NCAP):
                    parts = []
                    for i in range(NSPLIT):
                        pt = pg.tile([P, DIM], BF, tag=f"ebp{i}", bufs=2)
                        nc.sync.dma_start(out=pt[:], in_=ebufs[i][_ts(s, P), :])
                        parts.append(pt)
                    nc.vector.tensor_tensor(out=parts[0][:], in0=parts[0][:],
                                            in1=parts[1][:], op=OP.add)
                    nc.vector.tensor_tensor(out=parts[2][:], in0=parts[2][:],
                                            in1=parts[3][:], op=OP.add)
                    eb = pg.tile([P, DIM], BF, tag="eb")
                    nc.vector.tensor_tensor(out=eb[:], in0=parts[0][:],
                                            in1=parts[2][:], op=OP.add)
                    for j in range(ND):
                        pst = pg_ps.tile([P, P], BF, space="PSUM", tag="pstG")
                        nc.tensor.transpose(out=pst[:], in_=eb[:, _ts(j, P)],
                                            identity=id_bf[:])
                        nc.vector.tensor_copy(out=ebT[j][:, _ts(s, P)], in_=pst[:])
            e1_sb = load_w(pgw, ew1_in, "e1")
            e3_sb = load_w(pgw, ew3_in, "e3")
            gT = [pg_gT.tile([P, C], BF, name=f"gT{j}", tag=f"gT{j}")
                  for j in range(ND)]
            nsub = (C + 511) // 512
            with tc.tile_pool(name="stG_ps2", bufs=2, space="PSUM") as pg_ps2:
                for j in range(ND):
                    for s in range(nsub):
                        w = min(512, C - s * 512)
                        sl = slice(s * 512, s * 512 + w)
                        h1 = pg_ps2.tile([P, 512], F32, space="PSUM", tag="h1")
                        h3 = pg_ps2.tile([P, 512], F32, space="PSUM", tag="h3")
                        for d in range(ND):
                            nc.tensor.matmul(out=h1[:, :w],
                                             lhsT=e1_sb[d][:, _ts(j, P)],
                                             rhs=ebT[d][:, sl],
                                             start=(d == 0), stop=(d == ND - 1))
                        for d in range(ND):
                            nc.tensor.matmul(out=h3[:, :w],
                                             lhsT=e3_sb[d][:, _ts(j, P)],
                                             rhs=ebT[d][:, sl],
                                             start=(d == 0), stop=(d == ND - 1))
                        sig = pg.tile([P, 512], F32, tag="sig")
                        nc.scalar.activation(out=sig[:, :w], in_=h1[:, :w],
                                             func=ACTF.Sigmoid)
                        nc.vector.tensor_tensor(out=sig[:, :w], in0=sig[:, :w],
                                                in1=h1[:, :w], op=OP.mult)
                        nc.vector.tensor_tensor(out=gT[j][:, sl], in0=sig[:, :w],
                                                in1=h3[:, :w], op=OP.mult)
                e2_sb = load_w(pgw, ew2_in, "e2", slot="e1")
                for s in range(NCAP):
                    ps = pg_ps2.tile([P, DIM], F32, space="PSUM", tag="eops")
                    for half in range(2):
                        for j in range(ND):
                            nc.tensor.matmul(
                                out=ps[:, _ts(half, 512)],
                                lhsT=gT[j][:, _ts(s, P)],
                                rhs=e2_sb[j][:, _ts(half, 512)],
                                start=(j == 0), stop=(j == ND - 1))
                    eo = pg.tile([P, DIM], F32, tag="eo")
                    nc.vector.tensor_copy(out=eo[:], in_=ps[:])
                    nc.sync.dma_start(out=eo_out[_ts(s, P), :], in_=eo[:])
        nc.leave_named_scope("G_expert", scG[0], False)

    nc.compile()
    return nc


# ----------------------------------------------------------------------
# host side
# ----------------------------------------------------------------------

def prep_inputs(x, freqs, att_norm_w, wq, wk, wv, wo, ffn_norm_w, gate_w,
                ew1, ew2, ew3, sw1, sw2, sw3, LQ=LQ_FULL, n_cores=8):
    """Build the 8 per-core input maps (host-side weight folding + slicing)."""
    def tobf(a):
        return np.ascontiguousarray(np.asarray(a, np.float32).astype(BF16))

    B, S, _ = x.shape
    anw = np.asarray(att_norm_w, np.float32)
    fnw = np.asarray(ffn_norm_w, np.float32)
    wq_e = tobf((anw[:, None] * wq) / np.sqrt(HD))
    wk_e = tobf(anw[:, None] * wk)
    wv_e = tobf(anw[:, None] * wv)
    wo_e = tobf(wo)
    gate32 = np.ascontiguousarray((np.asarray(gate_w, np.float32) * fnw[None, :]).T)
    ew1_e = tobf(np.asarray(ew1) * fnw[None, :, None])
    ew3_e = tobf(np.asarray(ew3) * fnw[None, :, None])
    ew2_e = tobf(ew2)
    sw1_e = tobf(np.asarray(sw1) * fnw[:, None])
    sw3_e = tobf(np.asarray(sw3) * fnw[:, None])
    sw2_e = tobf(sw2)
    cosr = np.tile(np.asarray(freqs[:S, :, 0], np.float32), (1, NH))
    sinr = np.tile(np.asarray(freqs[:S, :, 1], np.float32), (1, NH))

    cpb = n_cores // B
    in_maps = []
    for core in range(n_cores):
        b = core // cpb
        qoff = (core % cpb) * LQ
        na = LQ * n_cores // 128
        oh = np.zeros((1, E), np.float32)
        oh[0, core % E] = 1.0
        oh = np.tile(oh, (1, na))
        in_maps.append(dict(
            x_chunk=np.ascontiguousarray(np.asarray(x[b, qoff:qoff + LQ], np.float32)),
            cos_chunk=np.ascontiguousarray(cosr[qoff:qoff + LQ]),
            sin_chunk=np.ascontiguousarray(sinr[qoff:qoff + LQ]),
            wq_bf=wq_e, wk_bf=wk_e, wv_bf=wv_e, wo_bf=wo_e,
            gate32=gate32,
            sw1_bf=sw1_e, sw2_bf=sw2_e, sw3_bf=sw3_e,
            ew1_bf=ew1_e[core % E], ew2_bf=ew2_e[core % E],
            ew3_bf=ew3_e[core % E],
            onehot=oh,
        ))
    return in_maps


def assemble(results, B, S, LQ=LQ_FULL, n_cores=8):
    N = B * S
    out = np.zeros((N, DIM), np.float32)
    y = np.zeros((N, DIM), np.float32)
    # replicate the device's top-2 selection exactly from the fp32 logits
    lg = np.asarray(results[0]["lg_out"], np.float32)          # (N, E)
    m2 = np.partition(lg, -2, axis=1)[:, -2]
    sel_mask = lg >= m2[:, None]
    ex = np.exp(lg - lg.max(axis=1, keepdims=True), dtype=np.float32)
    probs = ex / ex.sum(axis=1, keepdims=True, dtype=np.float32)
    cpb = n_cores // B
    for core, res in enumerate(results):
        b = core // cpb
        qoff = (core % cpb) * LQ
        tok0 = b * S + qoff
        out[tok0:tok0 + LQ] = res["out_local"]
        e = core % E
        sel = np.nonzero(sel_mask[:, e])[0]
        cnt = len(sel)
        eo = res["eo_out"]
        assert cnt <= eo.shape[0], (core, cnt)
        y[sel] += probs[sel, e:e + 1] * eo[:cnt]
    return (out + y).reshape(B, S, DIM)


_NC_CACHE = {}


def kernel(**inputs):
    key = "full"
    if key not in _NC_CACHE:
        _NC_CACHE[key] = build_nc()
    nc = _NC_CACHE[key]
    from concourse.bass_utils import run_bass_kernel_spmd
    in_maps = prep_inputs(**inputs)
    res = run_bass_kernel_spmd(nc, in_maps, core_ids=list(range(8)))
    x = np.asarray(inputs["x"])
    return assemble(res.results, x.shape[0], x.shape[1]).astype(np.float32)


if __name__ == "__main__":
    nc = build_nc()
    print("built + compiled OK")

